# revision 1
# baseline (speedup 1.0000x reference)
"""Self-contained Bass/Trainium2 kernel for nn_Attention (B=4, N=2048, D=1024, H=16, dh=64).

Sharding: 8 cores = (batch b in 0..3) x (sequence half in 0..1).
Each core computes the full attention output for its 1024 rows of its batch:
full-sequence K/V are computed on-core (duplicated across the pair), so no
cross-core communication is needed. Host feeds x[b] with the core's own rows
last so one SPMD program serves all cores; softmax is order-invariant in j.

Numerics: matmuls in float32r (TF32-class single-pass mode), attention
weights and V in fp16, accumulation in fp32 PSUM. Softmax uses the
exp-sum-divide form without max subtraction (scores are O(1) here); the row
sums come free from a ones-column appended to V, and the 1/sum normalization
is fused into the PSUM-evacuation multiply.
"""

import sys
import numpy as np

sys.path.insert(0, "/opt/trn_rl_repo")

B, N, DIM = 4, 2048, 1024
HEADS, DH = 16, 64
SCALE = DH ** -0.5  # 0.125
NC = 8
HALF = N // 2  # rows per core

_compiled = None


def _build():
    import concourse.tile as tile
    from concourse import bacc, mybir
    from concourse.masks import make_identity

    f32 = mybir.dt.float32
    f32r = mybir.dt.float32r
    f16 = mybir.dt.float16
    EXP = mybir.ActivationFunctionType.Exp

    nc = bacc.Bacc("TRN2", target_bir_lowering=False, debug=False, num_devices=NC)

    X = nc.dram_tensor("x", (DIM, N), f32r, kind="ExternalInput").ap()
    WQKV = nc.dram_tensor("w_qkv", (DIM, 3 * DIM), f32r, kind="ExternalInput").ap()
    WOUT = nc.dram_tensor("w_out", (DIM, DIM), f32r, kind="ExternalInput").ap()
    BOUT = nc.dram_tensor("b_out", (DIM,), f32, kind="ExternalInput").ap()
    Y = nc.dram_tensor("y", (HALF, DIM), f32, kind="ExternalOutput").ap()

    CT = DIM // 128          # 8 contraction tiles over channels
    MT = DIM // 128          # 8 dim tiles for each of q,k,v
    ICH = 512                # i-chunk width for attention
    NCH = HALF // ICH        # 2 chunks
    JT = N // 128            # 16 j tiles
    SEC = 2                  # j-tiles per PSUM/exp section
    NSEC = JT // SEC         # 8 sections
    VW = DH + 1              # 65: v block width incl. ones column

    with tile.TileContext(nc) as tc:
        with tc.tile_pool(name="persist", bufs=1) as persist, \
             tc.tile_pool(name="wpool", bufs=3) as wpool:

            ident32 = persist.tile([128, 128], f32, tag="ident32")
            make_identity(nc, ident32[:])
            ident = persist.tile([128, 128], f32r, tag="ident")
            nc.vector.tensor_copy(ident[:], ident32[:])

            # bias broadcast to all partitions once
            bias_src = persist.tile([1, DIM], f32, tag="bias_src")
            nc.sync.dma_start(bias_src[:], BOUT.rearrange("(o d) -> o d", o=1))
            bias = persist.tile([128, DIM], f32, tag="bias")
            nc.gpsimd.partition_broadcast(bias[:], bias_src[0:1, :])

            kT = [persist.tile([128, N], f32r, tag="kT", bufs=MT, name=f"kT{m}")
                  for m in range(MT)]
            v_ext = [persist.tile([128, HEADS * VW], f16, tag="vext", bufs=JT,
                                  name=f"vext{t}") for t in range(JT)]
            qT = [persist.tile([128, HALF], f32r, tag="qT", bufs=MT, name=f"qT{m}")
                  for m in range(MT)]

            def w_col(base, m):
                """[128, 8, 128] view of w_qkv[:, base+m*128 : +128], channel tiles in free."""
                return WQKV[:, base + m * 128:base + (m + 1) * 128].rearrange(
                    "(t p) d -> p t d", p=128)

            def project(psA, wApool, xbT_h, w_base, m):
                """One [128, HALF] psum tile of W[:, m-block].T @ xT for this half."""
                wt = wApool.tile([128, DIM], f32r, tag="wA", name=f"wt{w_base}_{m}")
                nc.sync.dma_start(wt.rearrange("p (t d) -> p t d", d=128), w_col(w_base, m))
                ps = psA.tile([128, HALF], f32, tag="proj", bufs=2, name=f"proj{w_base}_{m}")
                for ct in range(CT):
                    for s in range(HALF // 512):
                        nc.tensor.matmul(ps[:, s * 512:(s + 1) * 512],
                                         wt[:, ct * 128:(ct + 1) * 128],
                                         xbT_h[ct][:, s * 512:(s + 1) * 512],
                                         start=(ct == 0), stop=(ct == CT - 1))
                return ps

            with tc.tile_pool(name="psA", bufs=1, space="PSUM") as psA, \
                 tc.tile_pool(name="wApool", bufs=2) as wApool, \
                 tc.tile_pool(name="stage", bufs=2) as stage:
                for h in (0, 1):
                    # ---- x^T half h arrives pre-transposed from the host shard ----
                    xbT_h = [stage.tile([128, HALF], f32r, tag="xbT", bufs=CT,
                                        name=f"xbT{h}_{ct}") for ct in range(CT)]
                    for ct in range(CT):
                        nc.sync.dma_start(
                            xbT_h[ct][:],
                            X[ct * 128:(ct + 1) * 128, h * HALF:(h + 1) * HALF])

                    # ---- kT projection ----
                    for m in range(MT):
                        ps = project(psA, wApool, xbT_h, DIM, m)
                        nc.vector.tensor_copy(kT[m][:, h * HALF:(h + 1) * HALF], ps[:])

                    # ---- v projection (dims-major), then transpose into v_ext (fp16) ----
                    for m in range(MT):
                        ps = project(psA, wApool, xbT_h, 2 * DIM, m)
                        vs = stage.tile([128, HALF], f32r, tag="vstage")
                        nc.vector.tensor_copy(vs[:], ps[:])
                        for nt in range(HALF // 128):
                            tp = psA.tile([128, 128], f32r, tag="tp", bufs=4)
                            nc.tensor.transpose(tp[:], vs[:, nt * 128:(nt + 1) * 128],
                                                ident[:])
                            dst = v_ext[h * (HALF // 128) + nt].rearrange(
                                "p (hh c) -> p hh c", c=VW)[:, 2 * m:2 * m + 2, 0:DH]
                            nc.vector.tensor_copy(dst, tp.rearrange("p (hh c) -> p hh c", c=DH))

                    # ---- qT projection last, so attention can start right after ----
                    if h == 1:
                        for m in range(MT):
                            ps = project(psA, wApool, xbT_h, 0, m)
                            nc.vector.tensor_copy(qT[m][:], ps[:])

                # ones columns of v_ext
                for t in range(JT):
                    ones_col = v_ext[t].rearrange("p (hh c) -> p hh c", c=VW)[:, :, DH:VW]
                    nc.gpsimd.memset(ones_col, 1.0)

            # ================= phase B: attention + output projection =================
            with tc.tile_pool(name="attn", bufs=1) as attn, \
                 tc.tile_pool(name="psB", bufs=1, space="PSUM") as psB:
                for ch in range(NCH):
                    isl = slice(ch * ICH, (ch + 1) * ICH)
                    ctx = [attn.tile([128, ICH], f32r, tag="ctx", bufs=12,
                                     name=f"ctx{ch}_{t}") for t in range(MT)]
                    for hp in range(HEADS // 2):
                        po = [psB.tile([65, ICH], f32, tag="po", bufs=2,
                                       name=f"po{ch}_{hp}_{p}") for p in range(2)]
                        ats = {}
                        # software pipeline: dots(sec) -> exp(sec); av(sec-1) after
                        # dots(sec) so the PE never head-of-line blocks on ACT.
                        for sec in range(NSEC):
                            pp = [psB.tile([128, SEC * 512], f32, tag="dots", bufs=3,
                                           name=f"dots{ch}_{hp}_{sec}_{p}")
                                  for p in range(2)]
                            for j2 in range(SEC):
                                jt = sec * SEC + j2
                                for p in range(2):
                                    nc.tensor.matmul(
                                        pp[p][:, j2 * 512:(j2 + 1) * 512],
                                        kT[hp][p * 64:(p + 1) * 64, jt * 128:(jt + 1) * 128],
                                        qT[hp][p * 64:(p + 1) * 64, isl],
                                        start=True, stop=True)
                            at = [attn.tile([128, SEC * 512], f16, tag="attnT", bufs=5,
                                            name=f"at{ch}_{hp}_{sec}_{p}")
                                  for p in range(2)]
                            for p in range(2):
                                nc.scalar.activation(at[p][:], pp[p][:], EXP,
                                                     bias=0.0, scale=SCALE)
                            ats[sec] = at

                            def av(s):
                                for j2 in range(SEC):
                                    jt = s * SEC + j2
                                    for p in range(2):
                                        hd = 2 * hp + p
                                        nc.tensor.matmul(
                                            po[p][:],
                                            v_ext[jt][:, hd * VW:(hd + 1) * VW],
                                            ats[s][p][:, j2 * 512:(j2 + 1) * 512],
                                            start=(jt == 0), stop=(jt == JT - 1))

                            if sec >= 1:
                                av(sec - 1)
                                del ats[sec - 1]
                        av(NSEC - 1)
                        # normalize by the ones-column sums, write into ctx (f32r)
                        for p in range(2):
                            rs = attn.tile([128, ICH], f32, tag="rs", bufs=4)
                            nc.vector.reciprocal(rs[0:1, :], po[p][64:65, :])
                            rb = attn.tile([128, ICH], f32, tag="rb", bufs=4)
                            nc.gpsimd.partition_broadcast(rb[:], rs[0:1, :])
                            nc.vector.tensor_mul(ctx[hp][p * 64:(p + 1) * 64, :],
                                                 po[p][0:64, :], rb[p * 64:(p + 1) * 64, :])
                    # out-projection for this chunk: yp holds both 512-wide e-halves
                    for ipair in range(ICH // 256):
                        yp = [psB.tile([128, DIM], f32, tag="dots", bufs=3,
                                       name=f"yp{ch}_{ipair}_{i}") for i in range(2)]
                        for ft in range(MT):
                            wo = wpool.tile([128, DIM], f32r, tag="wO",
                                            name=f"wo{ch}_{ipair}_{ft}")
                            nc.sync.dma_start(wo[:], WOUT[ft * 128:(ft + 1) * 128, :])
                            for i in range(2):
                                it = ipair * 2 + i
                                for ec in range(2):
                                    nc.tensor.matmul(
                                        yp[i][:, ec * 512:(ec + 1) * 512],
                                        ctx[ft][:, it * 128:(it + 1) * 128],
                                        wo[:, ec * 512:(ec + 1) * 512],
                                        start=(ft == 0), stop=(ft == MT - 1))
                        for i in range(2):
                            it = ipair * 2 + i
                            for ec in range(2):
                                ysb = attn.tile([128, 512], f32, tag="ysb", bufs=2)
                                nc.vector.tensor_add(ysb[:], yp[i][:, ec * 512:(ec + 1) * 512],
                                                     bias[:, ec * 512:(ec + 1) * 512])
                                nc.sync.dma_start(
                                    Y[ch * ICH + it * 128:ch * ICH + (it + 1) * 128,
                                      ec * 512:(ec + 1) * 512], ysb[:])

    nc.compile()
    return nc


def _get_compiled():
    global _compiled
    if _compiled is None:
        _compiled = _build()
    return _compiled


def kernel(x, w_qkv, w_out, b_out):
    from concourse.bass_utils import run_bass_kernel_spmd

    nc = _get_compiled()
    x = np.asarray(x, dtype=np.float32)
    w_qkv = np.ascontiguousarray(np.asarray(w_qkv, dtype=np.float32))
    w_out = np.ascontiguousarray(np.asarray(w_out, dtype=np.float32))
    b_out = np.asarray(b_out, dtype=np.float32)

    in_maps = []
    for c in range(NC):
        b, half = divmod(c, 2)
        other = x[b][(1 - half) * HALF:(2 - half) * HALF]
        mine = x[b][half * HALF:(half + 1) * HALF]
        xb = np.ascontiguousarray(np.concatenate([other, mine], axis=0).T)
        in_maps.append({"x": xb, "w_qkv": w_qkv, "w_out": w_out, "b_out": b_out})

    res = run_bass_kernel_spmd(nc, in_maps, core_ids=list(range(NC)))

    out = np.empty((B, N, DIM), dtype=np.float32)
    for c in range(NC):
        b, half = divmod(c, 2)
        out[b, half * HALF:(half + 1) * HALF] = res.results[c]["y"]
    return out



# revision 10
# speedup vs baseline: 1.3693x; 1.3693x over previous
"""Self-contained Bass/Trainium2 kernel for nn_Attention (B=4, N=2048, D=1024, H=16, dh=64).

Sharding: 8 cores = (batch b in 0..3) x (sequence half in 0..1).
Each core computes the full attention output for its 1024 rows of its batch:
full-sequence K/V are computed on-core (duplicated across the pair), so no
cross-core communication is needed. Host feeds x[b]^T with the core's own rows
last so one SPMD program serves all cores; softmax is order-invariant in j.

v2 layout: all matmul operands fp16 (PSUM f32). V is projected directly in
keys-major layout (stationary = x^T blocks, moving = Wv) so no PE transposes
are needed. Each V block carries 64 ones columns, so the AV matmul emits the
softmax row-sums replicated across PSUM partitions 64..127 for free; the
normalization is then one reciprocal_approx_fast + one multiply on DVE.
Projection work is split into PSUM-tile-sized units and interleaved into the
attention loop at key-tile granularity, keeping the PE continuously busy (and
ramped) while the ACT engine streams the exps.
"""

import sys
import numpy as np

sys.path.insert(0, "/opt/trn_rl_repo")

B, N, DIM = 4, 2048, 1024
HEADS, DH = 16, 64
SCALE = DH ** -0.5  # 0.125
NC = 8
HALF = N // 2  # rows per core

_compiled = None


def _build():
    import concourse.tile as tile
    from concourse import bacc, mybir

    f32 = mybir.dt.float32
    f16 = mybir.dt.float16
    EXP = mybir.ActivationFunctionType.Exp

    nc = bacc.Bacc("TRN2", target_bir_lowering=False, debug=False, num_devices=NC)

    X = nc.dram_tensor("x", (DIM, N), f16, kind="ExternalInput").ap()
    WQKV = nc.dram_tensor("w_qkv", (DIM, 3 * DIM), f16, kind="ExternalInput").ap()
    WOUT = nc.dram_tensor("w_out", (DIM, DIM), f16, kind="ExternalInput").ap()
    BOUT = nc.dram_tensor("b_out", (DIM,), f32, kind="ExternalInput").ap()
    Y = nc.dram_tensor("y", (HALF, DIM), f32, kind="ExternalOutput").ap()

    CT = DIM // 128   # 8 contraction tiles over channels
    MT = DIM // 128   # 8 dim tiles (head-pairs) for each of q,k
    JT = N // 128     # 16 key tiles
    VW = 128          # per-head v block: 64 dims + 64 ones columns

    with tile.TileContext(nc) as tc:
        with tc.tile_pool(name="persist", bufs=1) as persist, \
             tc.tile_pool(name="attnbuf", bufs=1) as attnbuf, \
             tc.tile_pool(name="wpool", bufs=4) as wpool:

            kT = [persist.tile([128, N], f16, tag="kT", bufs=MT, name=f"kT{m}")
                  for m in range(MT)]
            qT = [persist.tile([128, HALF], f16, tag="qT", bufs=MT, name=f"qT{m}")
                  for m in range(MT)]
            v_ext = [persist.tile([128, HEADS * VW], f16, tag="vext", bufs=JT,
                                  name=f"vext{t}") for t in range(JT)]
            ctx = [persist.tile([128, HALF], f16, tag="ctx", bufs=MT,
                                name=f"ctx{m}") for m in range(MT)]

            # bias broadcast to all partitions once
            bias_src = persist.tile([1, DIM], f32, tag="bias_src")
            nc.sync.dma_start(bias_src[:], BOUT.rearrange("(o d) -> o d", o=1))
            bias = persist.tile([128, DIM], f32, tag="bias")
            nc.gpsimd.partition_broadcast(bias[:], bias_src[0:1, :])

            # prefire the exp table load off the critical path
            dummy = attnbuf.tile([1, 8], f16, tag="dummy")
            nc.scalar.activation(dummy[:], bias_src[0:1, 0:8], EXP,
                                 bias=0.0, scale=1.0)

            # ones columns of v_ext (disjoint from the V-projection writes)
            for t in range(JT):
                ones_col = v_ext[t].rearrange("p (hh c) -> p hh c", c=VW)[:, :, DH:VW]
                nc.gpsimd.memset(ones_col, 1.0)

            with tc.tile_pool(name="psB", bufs=1, space="PSUM") as psB, \
                 tc.tile_pool(name="stage", bufs=1) as stage, \
                 tc.tile_pool(name="psInt", bufs=1, space="PSUM") as psInt:
                # x^T tiles (both halves) and Wv, resident through the last
                # projection unit
                xbT = [[stage.tile([128, HALF], f16, tag="xbT", bufs=2 * CT,
                                   name=f"xbT{h}_{ct}") for h in (0, 1)]
                       for ct in range(CT)]
                for ct in range(CT):
                    for h in (0, 1):
                        nc.sync.dma_start(
                            xbT[ct][h][:],
                            X[ct * 128:(ct + 1) * 128, h * HALF:(h + 1) * HALF])
                wv = [stage.tile([128, DIM], f16, tag="wv", bufs=CT,
                                 name=f"wv{ct}") for ct in range(CT)]
                for ct in range(CT):
                    nc.sync.dma_start(
                        wv[ct][:], WQKV[ct * 128:(ct + 1) * 128, 2 * DIM:3 * DIM])

                def w_col(base, m):
                    """[128, 8, 128] view of w_qkv[:, base+m*128 : +128]."""
                    return WQKV[:, base + m * 128:base + (m + 1) * 128].rearrange(
                        "(t p) d -> p t d", p=128)

                # ---- projection units: one PSUM-tile lifecycle each ----
                wt_cache = {}

                def kq_unit(base, m, h, s, dst, off):
                    """dst[:, off + s*512 : +512] = W[:, m-block].T @ x^T[h]."""
                    key = (base, m)
                    if key not in wt_cache:
                        wt = wpool.tile([128, CT, 128], f16, tag="wkq",
                                        name=f"w{base}_{m}")
                        nc.sync.dma_start(wt[:], w_col(base, m))
                        wt_cache[key] = wt
                    wt = wt_cache[key]
                    ps = psInt.tile([128, 512], f32, tag="pint", bufs=2,
                                    name=f"pi{base}_{m}_{h}_{s}")
                    for ct in range(CT):
                        nc.tensor.matmul(ps[:],
                                         wt[:, ct, :],
                                         xbT[ct][h][:, s * 512:(s + 1) * 512],
                                         start=(ct == 0), stop=(ct == CT - 1))
                    nc.vector.tensor_copy(
                        dst[:, off + s * 512:off + (s + 1) * 512], ps[:])

                def v_unit(jtg, g):
                    """v_ext[jtg] heads 4g..4g+3 from x^T block (keys-major)."""
                    h, kt = divmod(jtg, CT)
                    ps = psInt.tile([128, 512], f32, tag="pint", bufs=2,
                                    name=f"pv{jtg}_{g}")
                    for ct in range(CT):
                        nc.tensor.matmul(
                            ps[:, 0:256],
                            xbT[ct][h][:, kt * 128:(kt + 1) * 128],
                            wv[ct][:, g * 256:(g + 1) * 256],
                            start=(ct == 0), stop=(ct == CT - 1))
                    dst = v_ext[jtg].rearrange("p (hh c) -> p hh c", c=VW)[
                        :, 4 * g:4 * g + 4, 0:DH]
                    nc.vector.tensor_copy(dst, ps[:, 0:256].rearrange(
                        "p (hh c) -> p hh c", c=DH))

                def group_units(g):
                    """Projection units for heads 4g..4g+3 (kT pairs 2g, 2g+1)."""
                    units = []
                    for m in (2 * g, 2 * g + 1):
                        for h in (0, 1):
                            for s in (0, 1):
                                units.append(lambda m=m, h=h, s=s: kq_unit(
                                    DIM, m, h, s, kT[m], h * HALF))
                    for jtg in range(JT):
                        units.append(lambda jtg=jtg, g=g: v_unit(jtg, g))
                    for m in (2 * g, 2 * g + 1):
                        for s in (0, 1):
                            units.append(lambda m=m, s=s: kq_unit(
                                0, m, 1, s, qT[m], 0))
                    return units

                pending = []

                # ---- attention for one head, interleaving pending units ----
                def attn_head(hd, interleave):
                    hp, p = divmod(hd, 2)
                    po = psB.tile([128, HALF], f32, tag="po", bufs=1,
                                  name=f"po{hd}")
                    ats = {}

                    def av(j):
                        at = ats.pop(j)
                        for s in (0, 1):
                            nc.tensor.matmul(
                                po[:, s * 512:(s + 1) * 512],
                                v_ext[j][:, hd * VW:(hd + 1) * VW],
                                at[:, s * 512:(s + 1) * 512],
                                start=(j == 0), stop=(j == JT - 1))

                    done = 0
                    for jt in range(JT):
                        pp = psB.tile([128, HALF], f32, tag="pp", bufs=2,
                                      name=f"pp{hd}_{jt}")
                        for s in (0, 1):
                            nc.tensor.matmul(
                                pp[:, s * 512:(s + 1) * 512],
                                kT[hp][p * 64:(p + 1) * 64,
                                       jt * 128:(jt + 1) * 128],
                                qT[hp][p * 64:(p + 1) * 64,
                                       s * 512:(s + 1) * 512],
                                start=True, stop=True)
                        at = attnbuf.tile([128, HALF], f16, tag="at", bufs=3,
                                          name=f"at{hd}_{jt}")
                        nc.scalar.activation(at[:], pp[:], EXP,
                                             bias=0.0, scale=SCALE)
                        ats[jt] = at
                        if jt >= 1:
                            av(jt - 1)
                        if interleave:
                            want = ((hd % 4) * JT + jt + 1) * len(interleave) \
                                // (4 * JT)
                            while done < want and pending:
                                pending.pop(0)()
                                done += 1
                    av(JT - 1)
                    # normalize by the replicated ones-column sums
                    # (reciprocal_approx_fast mis-reads partition-offset PSUM
                    # APs, so stage the sums into SBUF partitions 0..63 first)
                    ss = attnbuf.tile([64, HALF], f32, tag="ss", bufs=1,
                                      name=f"ss{hd}")
                    nc.vector.tensor_copy(ss[:], po[64:128, :])
                    rb = attnbuf.tile([64, HALF], f32, tag="rb", bufs=1,
                                      name=f"rb{hd}")
                    nc.vector.reciprocal_approx_fast(rb[:], ss[:])
                    nc.vector.tensor_mul(ctx[hp][p * 64:(p + 1) * 64, :],
                                         po[0:64, :], rb[:])

                # P0 up front; P(g+1) threads through A(g)'s slots
                for u in group_units(0):
                    u()
                for g in range(4):
                    if g < 3:
                        pending.extend(group_units(g + 1))
                    ileave = list(pending)
                    for hd in range(4 * g, 4 * g + 4):
                        attn_head(hd, ileave)
                    while pending:
                        pending.pop(0)()

            # ================= output projection =================
            with tc.tile_pool(name="outbuf", bufs=1) as outbuf, \
                 tc.tile_pool(name="psOut", bufs=1, space="PSUM") as psOut:
                wo = [outbuf.tile([128, DIM], f16, tag="wo", bufs=CT,
                                  name=f"wo{ft}") for ft in range(CT)]
                for ft in range(CT):
                    nc.sync.dma_start(
                        wo[ft][:], WOUT[ft * 128:(ft + 1) * 128, :])
                for qt in range(HALF // 128):
                    yp = psOut.tile([128, DIM], f32, tag="yp", bufs=3,
                                    name=f"yp{qt}")
                    for ft in range(MT):
                        for s in (0, 1):
                            nc.tensor.matmul(
                                yp[:, s * 512:(s + 1) * 512],
                                ctx[ft][:, qt * 128:(qt + 1) * 128],
                                wo[ft][:, s * 512:(s + 1) * 512],
                                start=(ft == 0), stop=(ft == MT - 1))
                    ysb = outbuf.tile([128, DIM], f32, tag="ysb", bufs=2,
                                      name=f"ysb{qt}")
                    nc.vector.tensor_add(ysb[:], yp[:], bias[:])
                    nc.sync.dma_start(
                        Y[qt * 128:(qt + 1) * 128, :], ysb[:])

    nc.compile()
    return nc


def _get_compiled():
    global _compiled
    if _compiled is None:
        _compiled = _build()
    return _compiled


def _build_in_maps(x, w_qkv, w_out, b_out):
    x = np.asarray(x, dtype=np.float32)
    w_qkv = np.ascontiguousarray(np.asarray(w_qkv, dtype=np.float16))
    w_out = np.ascontiguousarray(np.asarray(w_out, dtype=np.float16))
    b_out = np.asarray(b_out, dtype=np.float32)

    in_maps = []
    for c in range(NC):
        b, half = divmod(c, 2)
        other = x[b][(1 - half) * HALF:(2 - half) * HALF]
        mine = x[b][half * HALF:(half + 1) * HALF]
        xb = np.ascontiguousarray(
            np.concatenate([other, mine], axis=0).T.astype(np.float16))
        in_maps.append({"x": xb, "w_qkv": w_qkv, "w_out": w_out, "b_out": b_out})
    return in_maps


def kernel(x, w_qkv, w_out, b_out):
    from concourse.bass_utils import run_bass_kernel_spmd

    nc = _get_compiled()
    in_maps = _build_in_maps(x, w_qkv, w_out, b_out)
    res = run_bass_kernel_spmd(nc, in_maps, core_ids=list(range(NC)))

    out = np.empty((B, N, DIM), dtype=np.float32)
    for c in range(NC):
        b, half = divmod(c, 2)
        out[b, half * HALF:(half + 1) * HALF] = res.results[c]["y"]
    return out


# revision 22
# speedup vs baseline: 1.4086x; 1.0287x over previous
"""Self-contained Bass/Trainium2 kernel for nn_Attention (B=4, N=2048, D=1024, H=16, dh=64).

Sharding: 8 cores = (batch b in 0..3) x (sequence half in 0..1).
Each core computes the full attention output for its 1024 rows of its batch:
full-sequence K/V are computed on-core (duplicated across the pair), so no
cross-core communication is needed. Host feeds x[b]^T with the core's own rows
last so one SPMD program serves all cores; softmax is order-invariant in j.

v2 layout: all matmul operands fp16 (PSUM f32). V is projected directly in
keys-major layout (stationary = x^T blocks, moving = Wv) so no PE transposes
are needed. Each V block carries 64 ones columns, so the AV matmul emits the
softmax row-sums replicated across PSUM partitions 64..127 for free; the
normalization is then one reciprocal_approx_fast + one multiply on DVE.
Projection work is split into PSUM-tile-sized units and interleaved into the
attention loop at key-tile granularity, keeping the PE continuously busy (and
ramped) while the ACT engine streams the exps.
"""

import sys
import numpy as np

sys.path.insert(0, "/opt/trn_rl_repo")

B, N, DIM = 4, 2048, 1024
HEADS, DH = 16, 64
SCALE = DH ** -0.5  # 0.125
NC = 8
HALF = N // 2  # rows per core

_compiled = None


def _build():
    import concourse.tile as tile
    from concourse import bacc, mybir

    f32 = mybir.dt.float32
    f16 = mybir.dt.float16
    f8 = mybir.dt.float8e4
    EXP = mybir.ActivationFunctionType.Exp
    DR = mybir.MatmulPerfMode.DoubleRow

    nc = bacc.Bacc("TRN2", target_bir_lowering=False, debug=False, num_devices=NC)

    X = nc.dram_tensor("x", (DIM, N), f16, kind="ExternalInput").ap()
    WQKV = nc.dram_tensor("w_qkv", (DIM, 3 * DIM), f16, kind="ExternalInput").ap()
    WOUT = nc.dram_tensor("w_out", (DIM, DIM), f16, kind="ExternalInput").ap()
    BOUT = nc.dram_tensor("b_out", (DIM,), f32, kind="ExternalInput").ap()
    Y = nc.dram_tensor("y", (HALF, DIM), f32, kind="ExternalOutput").ap()

    CT = DIM // 128   # 8 contraction tiles over channels
    MT = DIM // 128   # 8 dim tiles (head-pairs) for each of q,k
    JT = N // 128     # 16 key tiles
    VW = 128          # per-head v block: 64 dims + 64 ones columns

    with tile.TileContext(nc) as tc:
        with tc.tile_pool(name="persist", bufs=1) as persist, \
             tc.tile_pool(name="attnbuf", bufs=1) as attnbuf, \
             tc.tile_pool(name="wpool", bufs=4) as wpool:

            kT = [persist.tile([128, N], f16, tag="kT", bufs=MT, name=f"kT{m}")
                  for m in range(MT)]
            qT = [persist.tile([128, HALF], f16, tag="qT", bufs=MT,
                               name=f"qT{m}") for m in range(MT)]
            v_ext = [persist.tile([128, HEADS * VW], f16, tag="vext", bufs=JT,
                                  name=f"vext{t}") for t in range(JT)]
            ctx = [persist.tile([128, HALF], f16, tag="ctx", bufs=MT,
                                name=f"ctx{m}") for m in range(MT)]

            # bias broadcast to all partitions once
            bias_src = persist.tile([1, DIM], f32, tag="bias_src")
            nc.sync.dma_start(bias_src[:], BOUT.rearrange("(o d) -> o d", o=1))
            bias = persist.tile([128, DIM], f32, tag="bias")
            nc.gpsimd.partition_broadcast(bias[:], bias_src[0:1, :])

            # prefire the exp table load off the critical path
            dummy = attnbuf.tile([1, 8], f16, tag="dummy")
            nc.scalar.activation(dummy[:], bias_src[0:1, 0:8], EXP,
                                 bias=0.0, scale=1.0)

            # ones columns of v_ext (disjoint from the V-projection writes)
            for t in range(JT):
                ones_col = v_ext[t].rearrange("p (hh c) -> p hh c", c=VW)[:, :, DH:VW]
                nc.gpsimd.memset(ones_col, 1.0)

            with tc.tile_pool(name="psB", bufs=1, space="PSUM") as psB, \
                 tc.tile_pool(name="stage", bufs=1) as stage, \
                 tc.tile_pool(name="psInt", bufs=1, space="PSUM") as psInt:
                def w_col(base, m):
                    """[128, 8, 128] view of w_qkv[:, base+m*128 : +128]."""
                    return WQKV[:, base + m * 128:base + (m + 1) * 128].rearrange(
                        "(t p) d -> p t d", p=128)

                # ---- projection units: one PSUM-tile lifecycle each ----
                wt_cache = {}

                def get_wt(base, m):
                    key = (base, m)
                    if key not in wt_cache:
                        wt = wpool.tile([128, CT, 128], f16, tag="wkq",
                                        name=f"w{base}_{m}")
                        nc.sync.dma_start(wt[:], w_col(base, m))
                        wt_cache[key] = wt
                    return wt_cache[key]

                # group-0 weight tiles first so the first K unit isn't stuck
                # behind the bulk x/wv transfers in the DMA queues
                for m in (0, 1):
                    get_wt(DIM, m)
                # x^T tiles (both halves) and Wv, resident through the last
                # projection unit
                xbT = [[stage.tile([128, HALF], f16, tag="xbT", bufs=2 * CT,
                                   name=f"xbT{h}_{ct}") for h in (0, 1)]
                       for ct in range(CT)]
                for ct in range(CT):
                    nc.sync.dma_start(
                        xbT[ct][0][:],
                        X[ct * 128:(ct + 1) * 128, 0:HALF])
                for m in (0, 1):
                    get_wt(0, m)
                for ct in range(CT):
                    nc.sync.dma_start(
                        xbT[ct][1][:],
                        X[ct * 128:(ct + 1) * 128, HALF:N])
                wv = [stage.tile([128, DIM], f16, tag="wv", bufs=CT,
                                 name=f"wv{ct}") for ct in range(CT)]
                for ct in range(CT):
                    nc.sync.dma_start(
                        wv[ct][:], WQKV[ct * 128:(ct + 1) * 128, 2 * DIM:3 * DIM])

                def kq_unit(base, m, h, s, dst, off):
                    """dst[:, off + s*512 : +512] = W[:, m-block].T @ x^T[h]."""
                    wt = get_wt(base, m)
                    ps = psInt.tile([128, 512], f32, tag="pint", bufs=2,
                                    name=f"pi{base}_{m}_{h}_{s}")
                    for ct in range(CT):
                        nc.tensor.matmul(ps[:],
                                         wt[:, ct, :],
                                         xbT[ct][h][:, s * 512:(s + 1) * 512],
                                         start=(ct == 0), stop=(ct == CT - 1))
                    nc.vector.tensor_copy(
                        dst[:, off + s * 512:off + (s + 1) * 512], ps[:])

                def v_unit(jtg, g):
                    """v_ext[jtg] heads 4g..4g+3 from x^T block (keys-major)."""
                    h, kt = divmod(jtg, CT)
                    ps = psInt.tile([128, 512], f32, tag="pint", bufs=2,
                                    name=f"pv{jtg}_{g}")
                    for ct in range(CT):
                        nc.tensor.matmul(
                            ps[:, 0:256],
                            xbT[ct][h][:, kt * 128:(kt + 1) * 128],
                            wv[ct][:, g * 256:(g + 1) * 256],
                            start=(ct == 0), stop=(ct == CT - 1))
                    dst = v_ext[jtg].rearrange("p (hh c) -> p hh c", c=VW)[
                        :, 4 * g:4 * g + 4, 0:DH]
                    nc.vector.tensor_copy(dst, ps[:, 0:256].rearrange(
                        "p (hh c) -> p hh c", c=DH))

                def group_units(g):
                    """Projection units for heads 4g..4g+3 (kT pairs 2g, 2g+1)."""
                    units = []
                    for m in (2 * g, 2 * g + 1):
                        for h in (0, 1):
                            for s in (0, 1):
                                units.append(lambda m=m, h=h, s=s: kq_unit(
                                    DIM, m, h, s, kT[m], h * HALF))
                    for jtg in range(JT):
                        units.append(lambda jtg=jtg, g=g: v_unit(jtg, g))
                    for m in (2 * g, 2 * g + 1):
                        for s in (0, 1):
                            units.append(lambda m=m, s=s: kq_unit(
                                0, m, 1, s, qT[m], 0))
                    return units

                pending = []

                # ---- attention for one head, interleaving pending units ----
                def attn_head(hd, interleave):
                    hp, p = divmod(hd, 2)
                    po = psB.tile([128, HALF], f32, tag="po", bufs=1,
                                  name=f"po{hd}")
                    ats = {}

                    def av(j):
                        at = ats.pop(j)
                        for s in (0, 1):
                            nc.tensor.matmul(
                                po[:, s * 512:(s + 1) * 512],
                                v_ext[j][:, hd * VW:(hd + 1) * VW],
                                at[:, s * 512:(s + 1) * 512],
                                start=(j == 0), stop=(j == JT - 1))

                    for jt in range(JT):
                        pp = psB.tile([128, HALF], f32, tag="pp", bufs=2,
                                      name=f"pp{hd}_{jt}")
                        for s in (0, 1):
                            nc.tensor.matmul(
                                pp[:, s * 512:(s + 1) * 512],
                                kT[hp][p * 64:(p + 1) * 64,
                                       jt * 128:(jt + 1) * 128],
                                qT[hp][p * 64:(p + 1) * 64,
                                       s * 512:(s + 1) * 512],
                                start=True, stop=True)
                        at = attnbuf.tile([128, HALF], f16, tag="at", bufs=3,
                                          name=f"at{hd}_{jt}")
                        nc.scalar.activation(at[:], pp[:], EXP,
                                             bias=0.0, scale=SCALE)
                        ats[jt] = at
                        if jt >= 1:
                            av(jt - 1)
                        if interleave:
                            want = ((hd % 4) * JT + jt + 1) * interleave[0] \
                                // (4 * JT)
                            while interleave[0] - len(pending) < want and pending:
                                pending.pop(0)()
                    av(JT - 1)
                    # normalize by the replicated ones-column sums
                    # (reciprocal_approx_fast mis-reads partition-offset PSUM
                    # APs, so stage the sums into SBUF partitions 0..63 first)
                    ss = attnbuf.tile([64, HALF], f32, tag="ss", bufs=1,
                                      name=f"ss{hd}")
                    nc.vector.tensor_copy(ss[:], po[64:128, :])
                    rb = attnbuf.tile([64, HALF], f32, tag="rb", bufs=1,
                                      name=f"rb{hd}")
                    nc.vector.reciprocal_approx_fast(rb[:], ss[:])
                    nc.vector.tensor_mul(ctx[hp][p * 64:(p + 1) * 64, :],
                                         po[0:64, :], rb[:])

                # P0 up front; P(g+1) threads through A(g)'s slots
                for u in group_units(0):
                    u()
                for g in range(4):
                    if g < 3:
                        pending.extend(group_units(g + 1))
                    ileave = [len(pending)] if pending else None
                    for hd in range(4 * g, 4 * g + 4):
                        attn_head(hd, ileave)
                    while pending:
                        pending.pop(0)()

            # ================= output projection =================
            with tc.tile_pool(name="outbuf", bufs=1) as outbuf, \
                 tc.tile_pool(name="psOut", bufs=1, space="PSUM") as psOut:
                wo = [outbuf.tile([128, DIM], f16, tag="wo", bufs=CT,
                                  name=f"wo{ft}") for ft in range(CT)]
                for ft in range(CT):
                    nc.sync.dma_start(
                        wo[ft][:], WOUT[ft * 128:(ft + 1) * 128, :])
                for qt in range(HALF // 128):
                    yp = psOut.tile([128, DIM], f32, tag="yp", bufs=3,
                                    name=f"yp{qt}")
                    for ft in range(MT):
                        for s in (0, 1):
                            nc.tensor.matmul(
                                yp[:, s * 512:(s + 1) * 512],
                                ctx[ft][:, qt * 128:(qt + 1) * 128],
                                wo[ft][:, s * 512:(s + 1) * 512],
                                start=(ft == 0), stop=(ft == MT - 1))
                    ysb = outbuf.tile([128, DIM], f32, tag="ysb", bufs=2,
                                      name=f"ysb{qt}")
                    nc.vector.tensor_add(ysb[:], yp[:], bias[:])
                    nc.sync.dma_start(
                        Y[qt * 128:(qt + 1) * 128, :], ysb[:])

    nc.compile()
    return nc


def _get_compiled():
    global _compiled
    if _compiled is None:
        _compiled = _build()
    return _compiled


def _build_in_maps(x, w_qkv, w_out, b_out):
    x = np.asarray(x, dtype=np.float32)
    w_qkv = np.ascontiguousarray(np.asarray(w_qkv, dtype=np.float16))
    w_out = np.ascontiguousarray(np.asarray(w_out, dtype=np.float16))
    b_out = np.asarray(b_out, dtype=np.float32)

    in_maps = []
    for c in range(NC):
        b, half = divmod(c, 2)
        other = x[b][(1 - half) * HALF:(2 - half) * HALF]
        mine = x[b][half * HALF:(half + 1) * HALF]
        xb = np.ascontiguousarray(
            np.concatenate([other, mine], axis=0).T.astype(np.float16))
        in_maps.append({"x": xb, "w_qkv": w_qkv, "w_out": w_out, "b_out": b_out})
    return in_maps


def kernel(x, w_qkv, w_out, b_out):
    from concourse.bass_utils import run_bass_kernel_spmd

    nc = _get_compiled()
    in_maps = _build_in_maps(x, w_qkv, w_out, b_out)
    res = run_bass_kernel_spmd(nc, in_maps, core_ids=list(range(NC)))

    out = np.empty((B, N, DIM), dtype=np.float32)
    for c in range(NC):
        b, half = divmod(c, 2)
        out[b, half * HALF:(half + 1) * HALF] = res.results[c]["y"]
    return out


# revision 23
# speedup vs baseline: 1.4158x; 1.0051x over previous
"""Self-contained Bass/Trainium2 kernel for nn_Attention (B=4, N=2048, D=1024, H=16, dh=64).

Sharding: 8 cores = (batch b in 0..3) x (sequence half in 0..1).
Each core computes the full attention output for its 1024 rows of its batch:
full-sequence K/V are computed on-core (duplicated across the pair), so no
cross-core communication is needed. Host feeds x[b]^T with the core's own rows
last so one SPMD program serves all cores; softmax is order-invariant in j.

v2 layout: all matmul operands fp16 (PSUM f32). V is projected directly in
keys-major layout (stationary = x^T blocks, moving = Wv) so no PE transposes
are needed. Each V block carries 64 ones columns, so the AV matmul emits the
softmax row-sums replicated across PSUM partitions 64..127 for free; the
normalization is then one reciprocal_approx_fast + one multiply on DVE.
Projection work is split into PSUM-tile-sized units and interleaved into the
attention loop at key-tile granularity, keeping the PE continuously busy (and
ramped) while the ACT engine streams the exps.
"""

import sys
import numpy as np

sys.path.insert(0, "/opt/trn_rl_repo")

B, N, DIM = 4, 2048, 1024
HEADS, DH = 16, 64
SCALE = DH ** -0.5  # 0.125
NC = 8
HALF = N // 2  # rows per core

_compiled = None


def _build():
    import concourse.tile as tile
    from concourse import bacc, mybir

    f32 = mybir.dt.float32
    f16 = mybir.dt.float16
    f8 = mybir.dt.float8e4
    EXP = mybir.ActivationFunctionType.Exp
    DR = mybir.MatmulPerfMode.DoubleRow

    nc = bacc.Bacc("TRN2", target_bir_lowering=False, debug=False, num_devices=NC)

    X = nc.dram_tensor("x", (DIM, N), f16, kind="ExternalInput").ap()
    WQKV = nc.dram_tensor("w_qkv", (DIM, 3 * DIM), f16, kind="ExternalInput").ap()
    WOUT = nc.dram_tensor("w_out", (DIM, DIM), f16, kind="ExternalInput").ap()
    BOUT = nc.dram_tensor("b_out", (DIM,), f32, kind="ExternalInput").ap()
    Y = nc.dram_tensor("y", (HALF, DIM), f32, kind="ExternalOutput").ap()

    CT = DIM // 128   # 8 contraction tiles over channels
    MT = DIM // 128   # 8 dim tiles (head-pairs) for each of q,k
    JT = N // 128     # 16 key tiles
    VW = 128          # per-head v block: 64 dims + 64 ones columns

    with tile.TileContext(nc) as tc:
        with tc.tile_pool(name="persist", bufs=1) as persist, \
             tc.tile_pool(name="attnbuf", bufs=1) as attnbuf, \
             tc.tile_pool(name="wpool", bufs=4) as wpool:

            kT = [persist.tile([128, N], f16, tag="kT", bufs=MT, name=f"kT{m}")
                  for m in range(MT)]
            qT = [persist.tile([128, HALF], f16, tag="qT", bufs=MT,
                               name=f"qT{m}") for m in range(MT)]
            v_ext = [persist.tile([128, HEADS * VW], f16, tag="vext", bufs=JT,
                                  name=f"vext{t}") for t in range(JT)]
            ctx = [persist.tile([128, HALF], f16, tag="ctx", bufs=MT,
                                name=f"ctx{m}") for m in range(MT)]

            # bias broadcast to all partitions once
            bias_src = persist.tile([1, DIM], f32, tag="bias_src")
            nc.sync.dma_start(bias_src[:], BOUT.rearrange("(o d) -> o d", o=1))
            bias = persist.tile([128, DIM], f32, tag="bias")
            nc.gpsimd.partition_broadcast(bias[:], bias_src[0:1, :])

            # prefire the exp table load off the critical path
            dummy = attnbuf.tile([1, 8], f16, tag="dummy")
            nc.scalar.activation(dummy[:], bias_src[0:1, 0:8], EXP,
                                 bias=0.0, scale=1.0)

            # ones columns of v_ext (disjoint from the V-projection writes)
            for t in range(JT):
                ones_col = v_ext[t].rearrange("p (hh c) -> p hh c", c=VW)[:, :, DH:VW]
                nc.gpsimd.memset(ones_col, 1.0)

            with tc.tile_pool(name="psB", bufs=1, space="PSUM") as psB, \
                 tc.tile_pool(name="stage", bufs=1) as stage, \
                 tc.tile_pool(name="psInt", bufs=1, space="PSUM") as psInt:
                def w_col(base, m):
                    """[128, 8, 128] view of w_qkv[:, base+m*128 : +128]."""
                    return WQKV[:, base + m * 128:base + (m + 1) * 128].rearrange(
                        "(t p) d -> p t d", p=128)

                # ---- projection units: one PSUM-tile lifecycle each ----
                wt_cache = {}

                def get_wt(base, m):
                    key = (base, m)
                    if key not in wt_cache:
                        wt = wpool.tile([128, CT, 128], f16, tag="wkq",
                                        name=f"w{base}_{m}")
                        nc.sync.dma_start(wt[:], w_col(base, m))
                        wt_cache[key] = wt
                    return wt_cache[key]

                # group-0 weight tiles first so the first K unit isn't stuck
                # behind the bulk x/wv transfers in the DMA queues
                for m in (0, 1):
                    get_wt(DIM, m)
                # x^T tiles (both halves) and Wv, resident through the last
                # projection unit
                xbT = [[stage.tile([128, HALF], f16, tag="xbT", bufs=2 * CT,
                                   name=f"xbT{h}_{ct}") for h in (0, 1)]
                       for ct in range(CT)]
                for ct in range(CT):
                    nc.sync.dma_start(
                        xbT[ct][0][:],
                        X[ct * 128:(ct + 1) * 128, 0:HALF])
                for m in (0, 1):
                    get_wt(0, m)
                for ct in range(CT):
                    nc.sync.dma_start(
                        xbT[ct][1][:],
                        X[ct * 128:(ct + 1) * 128, HALF:N])
                wv = [stage.tile([128, DIM], f16, tag="wv", bufs=CT,
                                 name=f"wv{ct}") for ct in range(CT)]
                for ct in range(CT):
                    nc.sync.dma_start(
                        wv[ct][:], WQKV[ct * 128:(ct + 1) * 128, 2 * DIM:3 * DIM])

                def kq_unit(base, m, h, s, dst, off):
                    """dst[:, off + s*512 : +512] = W[:, m-block].T @ x^T[h]."""
                    wt = get_wt(base, m)
                    ps = psInt.tile([128, 512], f32, tag="pint", bufs=2,
                                    name=f"pi{base}_{m}_{h}_{s}")
                    for ct in range(CT):
                        nc.tensor.matmul(ps[:],
                                         wt[:, ct, :],
                                         xbT[ct][h][:, s * 512:(s + 1) * 512],
                                         start=(ct == 0), stop=(ct == CT - 1))
                    nc.vector.tensor_copy(
                        dst[:, off + s * 512:off + (s + 1) * 512], ps[:])

                def v_unit(jtg, dc):
                    """v_ext[jtg] heads 8dc..8dc+7 from x^T block (keys-major)."""
                    h, kt = divmod(jtg, CT)
                    ps = psInt.tile([128, 512], f32, tag="pint", bufs=2,
                                    name=f"pv{jtg}_{dc}")
                    for ct in range(CT):
                        nc.tensor.matmul(
                            ps[:],
                            xbT[ct][h][:, kt * 128:(kt + 1) * 128],
                            wv[ct][:, dc * 512:(dc + 1) * 512],
                            start=(ct == 0), stop=(ct == CT - 1))
                    dst = v_ext[jtg].rearrange("p (hh c) -> p hh c", c=VW)[
                        :, 8 * dc:8 * dc + 8, 0:DH]
                    nc.vector.tensor_copy(dst, ps.rearrange(
                        "p (hh c) -> p hh c", c=DH))

                def group_units(g):
                    """Projection units for heads 4g..4g+3 (kT pairs 2g, 2g+1).
                    V is projected in 512-wide chunks (8 heads), carried by
                    groups 0 and 2."""
                    units = []
                    for m in (2 * g, 2 * g + 1):
                        for h in (0, 1):
                            for s in (0, 1):
                                units.append(lambda m=m, h=h, s=s: kq_unit(
                                    DIM, m, h, s, kT[m], h * HALF))
                    if g in (0, 2):
                        for jtg in range(JT):
                            units.append(lambda jtg=jtg, dc=g // 2: v_unit(
                                jtg, dc))
                    for m in (2 * g, 2 * g + 1):
                        for s in (0, 1):
                            units.append(lambda m=m, s=s: kq_unit(
                                0, m, 1, s, qT[m], 0))
                    return units

                pending = []

                # ---- attention for one head, interleaving pending units ----
                def attn_head(hd, interleave):
                    hp, p = divmod(hd, 2)
                    po = psB.tile([128, HALF], f32, tag="po", bufs=1,
                                  name=f"po{hd}")
                    ats = {}

                    def av(j):
                        at = ats.pop(j)
                        for s in (0, 1):
                            nc.tensor.matmul(
                                po[:, s * 512:(s + 1) * 512],
                                v_ext[j][:, hd * VW:(hd + 1) * VW],
                                at[:, s * 512:(s + 1) * 512],
                                start=(j == 0), stop=(j == JT - 1))

                    for jt in range(JT):
                        pp = psB.tile([128, HALF], f32, tag="pp", bufs=2,
                                      name=f"pp{hd}_{jt}")
                        for s in (0, 1):
                            nc.tensor.matmul(
                                pp[:, s * 512:(s + 1) * 512],
                                kT[hp][p * 64:(p + 1) * 64,
                                       jt * 128:(jt + 1) * 128],
                                qT[hp][p * 64:(p + 1) * 64,
                                       s * 512:(s + 1) * 512],
                                start=True, stop=True)
                        at = attnbuf.tile([128, HALF], f16, tag="at", bufs=3,
                                          name=f"at{hd}_{jt}")
                        nc.scalar.activation(at[:], pp[:], EXP,
                                             bias=0.0, scale=SCALE)
                        ats[jt] = at
                        if jt >= 1:
                            av(jt - 1)
                        if interleave:
                            want = ((hd % 4) * JT + jt + 1) * interleave[0] \
                                // (4 * JT)
                            while interleave[0] - len(pending) < want and pending:
                                pending.pop(0)()
                    av(JT - 1)
                    # normalize by the replicated ones-column sums
                    # (reciprocal_approx_fast mis-reads partition-offset PSUM
                    # APs, so stage the sums into SBUF partitions 0..63 first)
                    ss = attnbuf.tile([64, HALF], f32, tag="ss", bufs=1,
                                      name=f"ss{hd}")
                    nc.vector.tensor_copy(ss[:], po[64:128, :])
                    rb = attnbuf.tile([64, HALF], f32, tag="rb", bufs=1,
                                      name=f"rb{hd}")
                    nc.vector.reciprocal_approx_fast(rb[:], ss[:])
                    nc.vector.tensor_mul(ctx[hp][p * 64:(p + 1) * 64, :],
                                         po[0:64, :], rb[:])

                # P0 up front; P(g+1) threads through A(g)'s slots
                for u in group_units(0):
                    u()
                for g in range(4):
                    if g < 3:
                        pending.extend(group_units(g + 1))
                    ileave = [len(pending)] if pending else None
                    for hd in range(4 * g, 4 * g + 4):
                        attn_head(hd, ileave)
                    while pending:
                        pending.pop(0)()

            # ================= output projection =================
            with tc.tile_pool(name="outbuf", bufs=1) as outbuf, \
                 tc.tile_pool(name="psOut", bufs=1, space="PSUM") as psOut:
                wo = [outbuf.tile([128, DIM], f16, tag="wo", bufs=CT,
                                  name=f"wo{ft}") for ft in range(CT)]
                for ft in range(CT):
                    nc.sync.dma_start(
                        wo[ft][:], WOUT[ft * 128:(ft + 1) * 128, :])
                for qt in range(HALF // 128):
                    yp = psOut.tile([128, DIM], f32, tag="yp", bufs=3,
                                    name=f"yp{qt}")
                    for ft in range(MT):
                        for s in (0, 1):
                            nc.tensor.matmul(
                                yp[:, s * 512:(s + 1) * 512],
                                ctx[ft][:, qt * 128:(qt + 1) * 128],
                                wo[ft][:, s * 512:(s + 1) * 512],
                                start=(ft == 0), stop=(ft == MT - 1))
                    ysb = outbuf.tile([128, DIM], f32, tag="ysb", bufs=2,
                                      name=f"ysb{qt}")
                    nc.vector.tensor_add(ysb[:], yp[:], bias[:])
                    nc.sync.dma_start(
                        Y[qt * 128:(qt + 1) * 128, :], ysb[:])

    nc.compile()
    return nc


def _get_compiled():
    global _compiled
    if _compiled is None:
        _compiled = _build()
    return _compiled


def _build_in_maps(x, w_qkv, w_out, b_out):
    x = np.asarray(x, dtype=np.float32)
    w_qkv = np.ascontiguousarray(np.asarray(w_qkv, dtype=np.float16))
    w_out = np.ascontiguousarray(np.asarray(w_out, dtype=np.float16))
    b_out = np.asarray(b_out, dtype=np.float32)

    in_maps = []
    for c in range(NC):
        b, half = divmod(c, 2)
        other = x[b][(1 - half) * HALF:(2 - half) * HALF]
        mine = x[b][half * HALF:(half + 1) * HALF]
        xb = np.ascontiguousarray(
            np.concatenate([other, mine], axis=0).T.astype(np.float16))
        in_maps.append({"x": xb, "w_qkv": w_qkv, "w_out": w_out, "b_out": b_out})
    return in_maps


def kernel(x, w_qkv, w_out, b_out):
    from concourse.bass_utils import run_bass_kernel_spmd

    nc = _get_compiled()
    in_maps = _build_in_maps(x, w_qkv, w_out, b_out)
    res = run_bass_kernel_spmd(nc, in_maps, core_ids=list(range(NC)))

    out = np.empty((B, N, DIM), dtype=np.float32)
    for c in range(NC):
        b, half = divmod(c, 2)
        out[b, half * HALF:(half + 1) * HALF] = res.results[c]["y"]
    return out


# revision 31
# speedup vs baseline: 1.4364x; 1.0146x over previous
"""Self-contained Bass/Trainium2 kernel for nn_Attention (B=4, N=2048, D=1024, H=16, dh=64).

Sharding: 8 cores = (batch b in 0..3) x (sequence half in 0..1).
Each core computes the full attention output for its 1024 rows of its batch:
full-sequence K/V are computed on-core (duplicated across the pair), so no
cross-core communication is needed. Host feeds x[b]^T with the core's own rows
last so one SPMD program serves all cores; softmax is order-invariant in j.

v2 layout: all matmul operands fp16 (PSUM f32). V is projected directly in
keys-major layout (stationary = x^T blocks, moving = Wv) so no PE transposes
are needed. Each V block carries 64 ones columns, so the AV matmul emits the
softmax row-sums replicated across PSUM partitions 64..127 for free; the
normalization is then one reciprocal_approx_fast + one multiply on DVE.
Projection work is split into PSUM-tile-sized units and interleaved into the
attention loop at key-tile granularity, keeping the PE continuously busy (and
ramped) while the ACT engine streams the exps.
"""

import sys
import numpy as np

sys.path.insert(0, "/opt/trn_rl_repo")

B, N, DIM = 4, 2048, 1024
HEADS, DH = 16, 64
SCALE = DH ** -0.5  # 0.125
NC = 8
HALF = N // 2  # rows per core

_compiled = None


def _build():
    import concourse.tile as tile
    from concourse import bacc, mybir

    f32 = mybir.dt.float32
    f16 = mybir.dt.float16
    EXP = mybir.ActivationFunctionType.Exp

    nc = bacc.Bacc("TRN2", target_bir_lowering=False, debug=False, num_devices=NC)

    X = nc.dram_tensor("x", (DIM, N), f16, kind="ExternalInput").ap()
    WQKV = nc.dram_tensor("w_qkv", (DIM, 3 * DIM), f16, kind="ExternalInput").ap()
    WOUT = nc.dram_tensor("w_out", (DIM, DIM), f16, kind="ExternalInput").ap()
    BOUT = nc.dram_tensor("b_out", (DIM,), f32, kind="ExternalInput").ap()
    Y = nc.dram_tensor("y", (HALF, DIM), f32, kind="ExternalOutput").ap()

    CT = DIM // 128   # 8 contraction tiles over channels
    MT = DIM // 128   # 8 dim tiles (head-pairs) for each of q,k
    JT = N // 128     # 16 key tiles
    VW = 128          # per-head v block: 64 dims + 64 ones columns

    with tile.TileContext(nc) as tc:
        with tc.tile_pool(name="persist", bufs=1) as persist, \
             tc.tile_pool(name="attnbuf", bufs=1) as attnbuf, \
             tc.tile_pool(name="wpool", bufs=4) as wpool:

            kT = [persist.tile([128, N], f16, tag="kT", bufs=MT, name=f"kT{m}")
                  for m in range(MT)]
            qT = [persist.tile([128, HALF], f16, tag="qT", bufs=MT,
                               name=f"qT{m}") for m in range(MT)]
            v_ext = [persist.tile([128, HEADS * VW], f16, tag="vext", bufs=JT,
                                  name=f"vext{t}") for t in range(JT)]
            ctx = [persist.tile([128, HALF], f16, tag="ctx", bufs=MT,
                                name=f"ctx{m}") for m in range(MT)]

            # bias broadcast to all partitions once
            bias_src = persist.tile([1, DIM], f32, tag="bias_src")
            nc.sync.dma_start(bias_src[:], BOUT.rearrange("(o d) -> o d", o=1))
            bias = persist.tile([128, DIM], f32, tag="bias")
            nc.gpsimd.partition_broadcast(bias[:], bias_src[0:1, :])

            # prefire the exp table load off the critical path
            dummy = attnbuf.tile([1, 8], f16, tag="dummy")
            nc.scalar.activation(dummy[:], bias_src[0:1, 0:8], EXP,
                                 bias=0.0, scale=1.0)

            # ones columns of v_ext (disjoint from the V-projection writes)
            for t in range(JT):
                ones_col = v_ext[t].rearrange(
                    "p (hh c) -> p hh c", c=VW)[:, :, DH:VW]
                nc.gpsimd.memset(ones_col, 1.0)

            with tc.tile_pool(name="psB", bufs=1, space="PSUM") as psB, \
                 tc.tile_pool(name="stage", bufs=1) as stage, \
                 tc.tile_pool(name="psInt", bufs=1, space="PSUM") as psInt:
                def w_col(base, m):
                    """[128, 8, 128] view of w_qkv[:, base+m*128 : +128]."""
                    return WQKV[:, base + m * 128:base + (m + 1) * 128].rearrange(
                        "(t p) d -> p t d", p=128)

                # ---- projection units: one PSUM-tile lifecycle each ----
                wt_cache = {}

                def get_wt(base, m):
                    key = (base, m)
                    if key not in wt_cache:
                        wt = wpool.tile([128, CT, 128], f16, tag="wkq",
                                        name=f"w{base}_{m}")
                        nc.sync.dma_start(wt[:], w_col(base, m))
                        wt_cache[key] = wt
                    return wt_cache[key]

                # group-0 weight tiles first so the first K unit isn't stuck
                # behind the bulk x/wv transfers in the DMA queues
                for m in (0, 1):
                    get_wt(DIM, m)
                # x^T tiles (both halves) and Wv, resident through the last
                # projection unit
                xbT = [[stage.tile([128, HALF], f16, tag="xbT", bufs=2 * CT,
                                   name=f"xbT{h}_{ct}") for h in (0, 1)]
                       for ct in range(CT)]
                for ct in range(CT):
                    nc.sync.dma_start(
                        xbT[ct][0][:],
                        X[ct * 128:(ct + 1) * 128, 0:HALF])
                for m in (0, 1):
                    get_wt(0, m)
                for ct in range(CT):
                    nc.sync.dma_start(
                        xbT[ct][1][:],
                        X[ct * 128:(ct + 1) * 128, HALF:N])
                wv = [stage.tile([128, DIM], f16, tag="wv", bufs=CT,
                                 name=f"wv{ct}") for ct in range(CT)]
                for ct in range(CT):
                    nc.sync.dma_start(
                        wv[ct][:], WQKV[ct * 128:(ct + 1) * 128, 2 * DIM:3 * DIM])

                def kq_unit(base, m, h, s, dst, off):
                    """dst[:, off + s*512 : +512] = W[:, m-block].T @ x^T[h]."""
                    wt = get_wt(base, m)
                    ps = psInt.tile([128, 512], f32, tag="pint", bufs=2,
                                    name=f"pi{base}_{m}_{h}_{s}")
                    for ct in range(CT):
                        nc.tensor.matmul(ps[:],
                                         wt[:, ct, :],
                                         xbT[ct][h][:, s * 512:(s + 1) * 512],
                                         start=(ct == 0), stop=(ct == CT - 1))
                    nc.vector.tensor_copy(
                        dst[:, off + s * 512:off + (s + 1) * 512], ps[:])

                def v_unit(jtg, dc):
                    """v_ext[jtg] heads 8dc..8dc+7 from x^T block (keys-major)."""
                    h, kt = divmod(jtg, CT)
                    ps = psInt.tile([128, 512], f32, tag="pint", bufs=2,
                                    name=f"pv{jtg}_{dc}")
                    for ct in range(CT):
                        nc.tensor.matmul(
                            ps[:],
                            xbT[ct][h][:, kt * 128:(kt + 1) * 128],
                            wv[ct][:, dc * 512:(dc + 1) * 512],
                            start=(ct == 0), stop=(ct == CT - 1))
                    dst = v_ext[jtg].rearrange("p (hh c) -> p hh c", c=VW)[
                        :, 8 * dc:8 * dc + 8, 0:DH]
                    nc.vector.tensor_copy(dst, ps.rearrange(
                        "p (hh c) -> p hh c", c=DH))

                def group_units(g):
                    """Projection units for heads 4g..4g+3 (kT pairs 2g, 2g+1).
                    V is projected in 512-wide chunks (8 heads), carried by
                    groups 0 and 2."""
                    units = []
                    for m in (2 * g, 2 * g + 1):
                        for h in (0, 1):
                            for s in (0, 1):
                                units.append(lambda m=m, h=h, s=s: kq_unit(
                                    DIM, m, h, s, kT[m], h * HALF))
                    if g in (0, 2):
                        for jtg in range(JT):
                            units.append(lambda jtg=jtg, dc=g // 2: v_unit(
                                jtg, dc))
                    for m in (2 * g, 2 * g + 1):
                        for s in (0, 1):
                            units.append(lambda m=m, s=s: kq_unit(
                                0, m, 1, s, qT[m], 0))
                    return units

                pending = []

                # ---- attention for one head, interleaving pending units ----
                def attn_head(hd, interleave):
                    hp, p = divmod(hd, 2)
                    po = psB.tile([128, HALF], f32, tag="po", bufs=1,
                                  name=f"po{hd}")
                    ats = {}

                    def av(j):
                        at = ats.pop(j)
                        for s in (0, 1):
                            nc.tensor.matmul(
                                po[:, s * 512:(s + 1) * 512],
                                v_ext[j][:, hd * VW:(hd + 1) * VW],
                                at[:, s * 512:(s + 1) * 512],
                                start=(j == 0), stop=(j == JT - 1))

                    for jt in range(JT):
                        pp = psB.tile([128, HALF], f32, tag="pp", bufs=2,
                                      name=f"pp{hd}_{jt}")
                        for s in (0, 1):
                            nc.tensor.matmul(
                                pp[:, s * 512:(s + 1) * 512],
                                kT[hp][p * 64:(p + 1) * 64,
                                       jt * 128:(jt + 1) * 128],
                                qT[hp][p * 64:(p + 1) * 64,
                                       s * 512:(s + 1) * 512],
                                start=True, stop=True)
                        at = attnbuf.tile([128, HALF], f16, tag="at", bufs=3,
                                          name=f"at{hd}_{jt}")
                        nc.scalar.activation(at[:], pp[:], EXP,
                                             bias=0.0, scale=SCALE)
                        ats[jt] = at
                        if jt >= 1:
                            av(jt - 1)
                        if interleave:
                            want = ((hd % 4) * JT + jt + 1) * interleave[0] \
                                // (4 * JT)
                            while interleave[0] - len(pending) < want and pending:
                                pending.pop(0)()
                    av(JT - 1)
                    # normalize by the replicated ones-column sums
                    # (reciprocal_approx_fast mis-reads partition-offset PSUM
                    # APs, so stage the sums into SBUF partitions 0..63 first)
                    ss = attnbuf.tile([64, HALF], f32, tag="ss", bufs=1,
                                      name=f"ss{hd}")
                    nc.vector.tensor_copy(ss[:], po[64:128, :])
                    rb = attnbuf.tile([64, HALF], f32, tag="rb", bufs=1,
                                      name=f"rb{hd}")
                    nc.vector.reciprocal_approx_fast(rb[:], ss[:])
                    nc.vector.tensor_mul(ctx[hp][p * 64:(p + 1) * 64, :],
                                         po[0:64, :], rb[:])

                # P0 up front; P(g+1) threads through A(g)'s slots
                for u in group_units(0):
                    u()
                for g in range(3):
                    if g < 2:
                        pending.extend(group_units(g + 1))
                    else:
                        pending.extend(group_units(3))
                    ileave = [len(pending)] if pending else None
                    for hd in range(4 * g, 4 * g + 4):
                        attn_head(hd, ileave)
                    while pending:
                        pending.pop(0)()

            # stage (x^T, wv) is gone; w_out + the first half of the output
            # projection overlap A3, the tail reuses the psInt tiles
            with tc.tile_pool(name="outw", bufs=1) as outw:
                wo = [outw.tile([128, DIM], f16, tag="wo", bufs=CT,
                                name=f"wo{ft}") for ft in range(CT)]
                for ft in range(CT):
                    nc.sync.dma_start(
                        wo[ft][:], WOUT[ft * 128:(ft + 1) * 128, :])

                def yp_unit(qt, s, f0, src):
                    """dst s-half = src-half + sum(ft in f0..f0+3) ctx.T @ wo."""
                    ps = psInt.tile([128, 512], f32, tag="pint", bufs=2,
                                    name=f"py{qt}_{s}_{f0}")
                    for ft in range(f0, f0 + 4):
                        nc.tensor.matmul(ps[:],
                                         ctx[ft][:, qt * 128:(qt + 1) * 128],
                                         wo[ft][:, s * 512:(s + 1) * 512],
                                         start=(ft == f0), stop=(ft == f0 + 3))
                    sl = slice(s * 512, (s + 1) * 512)
                    if f0 == 0:
                        nc.vector.tensor_add(yps[qt][:, sl], ps[:], bias[:, sl])
                    else:
                        ysb = outw.tile([128, 512], f32, tag="ysb", bufs=3,
                                        name=f"ysb{qt}_{s}")
                        nc.vector.tensor_add(ysb[:], ps[:], yps[qt][:, sl])
                        nc.sync.dma_start(
                            Y[qt * 128:(qt + 1) * 128, sl], ysb[:])

                pending.extend(lambda qt=qt, s=s: yp_unit(qt, s, 0, None)
                               for qt in range(CT) for s in (0, 1))
                ileave = [len(pending)]
                for hd in range(12, 16):
                    attn_head(hd, ileave)
                while pending:
                    pending.pop(0)()
                # tail: second half of the output projection
                for qt in range(CT):
                    for s in (0, 1):
                        yp_unit(qt, s, 4, yps[qt])

    nc.compile()
    return nc


def _get_compiled():
    global _compiled
    if _compiled is None:
        _compiled = _build()
    return _compiled


def _build_in_maps(x, w_qkv, w_out, b_out):
    x = np.asarray(x, dtype=np.float32)
    w_qkv = np.ascontiguousarray(np.asarray(w_qkv, dtype=np.float16))
    w_out = np.ascontiguousarray(np.asarray(w_out, dtype=np.float16))
    b_out = np.asarray(b_out, dtype=np.float32)

    in_maps = []
    for c in range(NC):
        b, half = divmod(c, 2)
        other = x[b][(1 - half) * HALF:(2 - half) * HALF]
        mine = x[b][half * HALF:(half + 1) * HALF]
        xb = np.ascontiguousarray(
            np.concatenate([other, mine], axis=0).T.astype(np.float16))
        in_maps.append({"x": xb, "w_qkv": w_qkv, "w_out": w_out, "b_out": b_out})
    return in_maps


def kernel(x, w_qkv, w_out, b_out):
    from concourse.bass_utils import run_bass_kernel_spmd

    nc = _get_compiled()
    in_maps = _build_in_maps(x, w_qkv, w_out, b_out)
    res = run_bass_kernel_spmd(nc, in_maps, core_ids=list(range(NC)))

    out = np.empty((B, N, DIM), dtype=np.float32)
    for c in range(NC):
        b, half = divmod(c, 2)
        out[b, half * HALF:(half + 1) * HALF] = res.results[c]["y"]
    return out
